# revision 3
# baseline (speedup 1.0000x reference)
"""Trainium2 Bass kernel for SingleDGC (GCNConv + per-batch pairwise-distance
adjacency), data-parallel over 8 NeuronCores.

kernel(X, edge_index, W, b) -> (Xo [512,32,256] f32, adj_mask [512,32,32] bool)

Sharding: each core owns a contiguous block of 2048 target nodes (= 64
graphs). Edges are partitioned by target on the host (random edges cross
graph boundaries, so target-major partitioning is required), sorted by
target, and padded per 128-target window. On device, each core computes the
full XW = X @ W (from a host-transposed XT so lhsT needs no on-chip
transpose), scales rows by dinv[src], stages Y to DRAM, gathers per-edge
source rows with dma_gather, and scatter-adds them into PSUM with one-hot
matmuls (M^T @ Y). Stage 2 computes per-graph pairwise squared distances
with an augmented-matmul trick and thresholds the min-max normalized scores.
"""
import numpy as np

B, A, D = 512, 32, 256
N = B * A                 # 16384 nodes
NCORES = 8
TPC = N // NCORES         # 2048 targets per core
WIN = 128                 # targets per PSUM window
NW = TPC // WIN           # 16 windows per core
NWG = N // WIN            # 128 global windows
EPS = 1e-5
THRESHOLD = 0.5

_compiled = {}            # CPW -> (nc, input name list)


def _build(CPW):
    import concourse.bacc as bacc
    import concourse.tile as tile
    from concourse import mybir
    from concourse.masks import make_identity
    import contextlib

    dt = mybir.dt
    P = 128
    EPW = CPW * P                     # padded edges per window
    NCH = NW * CPW                    # chunks per core
    G0 = (CPW + 1) // 2               # chunks in first gather group

    nc = bacc.Bacc("TRN2", target_bir_lowering=False, debug=False,
                   num_devices=NCORES)

    XT = nc.dram_tensor("XT", [D, N], dt.float32, kind="ExternalInput").ap()
    Wm = nc.dram_tensor("Wm", [D, D], dt.float32, kind="ExternalInput").ap()
    b_rep = nc.dram_tensor("b_rep", [P, D], dt.float32, kind="ExternalInput").ap()
    dinv_full_t = nc.dram_tensor("dinv_full_t", [P, N // P], dt.float32,
                                 kind="ExternalInput").ap()
    dinv_t = nc.dram_tensor("dinv_t", [P, NW], dt.float32, kind="ExternalInput").ap()
    srcw = nc.dram_tensor("srcw", [P, NW * EPW // 16], dt.int16,
                          kind="ExternalInput").ap()
    tgtr = nc.dram_tensor("tgtr", [P, NCH], dt.float32, kind="ExternalInput").ap()

    Xo_out = nc.dram_tensor("Xo_out", [TPC, D], dt.float32, kind="ExternalOutput").ap()
    adj_out = nc.dram_tensor("adj_out", [TPC, A], dt.uint8, kind="ExternalOutput").ap()

    with tile.TileContext(nc) as tc:
        with contextlib.ExitStack() as ctx:
            const = ctx.enter_context(tc.tile_pool(name="const", bufs=1))
            xt_pool = ctx.enter_context(tc.tile_pool(name="xt", bufs=3))
            psum_xw = ctx.enter_context(tc.tile_pool(name="pxw", bufs=2, space="PSUM"))
            y_pool = ctx.enter_context(tc.tile_pool(name="y", bufs=3))
            dram = ctx.enter_context(tc.tile_pool(name="dram", bufs=1, space="DRAM"))
            gath_pool = ctx.enter_context(tc.tile_pool(name="gath", bufs=2))
            m_pool = ctx.enter_context(tc.tile_pool(name="m", bufs=4))
            psum_agg = ctx.enter_context(tc.tile_pool(name="pagg", bufs=2, space="PSUM"))
            xo_res = ctx.enter_context(tc.tile_pool(name="xores", bufs=1))
            psum_tp = ctx.enter_context(tc.tile_pool(name="ptp", bufs=2, space="PSUM"))
            psum_sq = ctx.enter_context(tc.tile_pool(name="psq", bufs=1, space="PSUM"))
            psum_sc = ctx.enter_context(tc.tile_pool(name="psc", bufs=1, space="PSUM"))
            s2_pool = ctx.enter_context(tc.tile_pool(name="s2", bufs=2))

            # ---- constants
            w0 = const.tile([P, D], dt.float32)
            nc.sync.dma_start(w0[:], Wm[0:P, :])
            w1 = const.tile([P, D], dt.float32)
            nc.sync.dma_start(w1[:], Wm[P:D, :])
            t_brep = const.tile([P, D], dt.float32)
            nc.sync.dma_start(t_brep[:], b_rep[:])
            t_dft = const.tile([P, N // P], dt.float32)
            nc.sync.dma_start(t_dft[:], dinv_full_t[:])
            t_dt = const.tile([P, NW], dt.float32)
            nc.sync.dma_start(t_dt[:], dinv_t[:])
            t_srcw = const.tile([P, NW * EPW // 16], dt.int16)
            nc.sync.dma_start(t_srcw[:], srcw[:])
            t_tgtr = const.tile([P, NCH], dt.float32)
            nc.sync.dma_start(t_tgtr[:], tgtr[:])
            iota_i = const.tile([P, P], dt.int32)
            nc.gpsimd.iota(iota_i[:], pattern=[[1, P]], base=0, channel_multiplier=0)
            iota_f = const.tile([P, P], dt.float32)
            nc.vector.tensor_copy(iota_f[:], iota_i[:])
            ident = const.tile([P, P], dt.float32)
            make_identity(nc, ident[:])
            ones_col = const.tile([P, 1], dt.float32)
            nc.vector.memset(ones_col[:], 1.0)

            Y_dram = dram.tile([N, D], dt.float32)

            # ---- phase 1: Y = (X @ W) * dinv[row], staged to DRAM
            for bt in range(N // 512):
                xt0 = xt_pool.tile([P, 512], dt.float32, tag="xt0")
                nc.sync.dma_start(xt0[:], XT[0:P, 512 * bt:512 * (bt + 1)])
                xt1 = xt_pool.tile([P, 512], dt.float32, tag="xt1")
                nc.sync.dma_start(xt1[:], XT[P:D, 512 * bt:512 * (bt + 1)])
                for j in range(4):
                    t = 4 * bt + j
                    pxw = psum_xw.tile([P, D], dt.float32)
                    nc.tensor.matmul(out=pxw[:], lhsT=xt0[:, P * j:P * (j + 1)],
                                     rhs=w0[:], start=True, stop=False)
                    nc.tensor.matmul(out=pxw[:], lhsT=xt1[:, P * j:P * (j + 1)],
                                     rhs=w1[:], start=False, stop=True)
                    y = y_pool.tile([P, D], dt.float32)
                    nc.vector.tensor_scalar(out=y[:], in0=pxw[:],
                                            scalar1=t_dft[:, t:t + 1], scalar2=None,
                                            op0=mybir.AluOpType.mult)
                    nc.sync.dma_start(Y_dram[P * t:P * (t + 1), :], y[:])

            # ---- phase 2: aggregation per window
            xo_sb = xo_res.tile([P, NW, D], dt.float32)
            for w in range(NW):
                pagg = psum_agg.tile([P, D], dt.float32)
                for (c0, c1) in ((0, G0), (G0, CPW)):
                    ng = c1 - c0
                    gt = gath_pool.tile([P, G0, D], dt.float32, tag="gt")
                    col0 = (w * CPW + c0) * 8
                    nc.gpsimd.dma_gather(gt[:, 0:ng, :], Y_dram[:],
                                         t_srcw[:, col0:col0 + ng * 8],
                                         ng * P, ng * P, D, single_packet=False)
                    for c in range(c0, c1):
                        m = m_pool.tile([P, P], dt.float32)
                        ci = w * CPW + c
                        nc.vector.tensor_scalar(out=m[:], in0=iota_f[:],
                                                scalar1=t_tgtr[:, ci:ci + 1],
                                                scalar2=None,
                                                op0=mybir.AluOpType.is_equal)
                        nc.tensor.matmul(out=pagg[:], lhsT=m[:], rhs=gt[:, c - c0, :],
                                         start=(c == 0), stop=(c == CPW - 1))
                # xo = pagg * dinv[tgt] + b
                nc.vector.scalar_tensor_tensor(out=xo_sb[:, w, :], in0=pagg[:],
                                               scalar=t_dt[:, w:w + 1],
                                               in1=t_brep[:],
                                               op0=mybir.AluOpType.mult,
                                               op1=mybir.AluOpType.add)
                nc.sync.dma_start(Xo_out[P * w:P * (w + 1), :], xo_sb[:, w, :])

            # ---- phase 3: per-graph pairwise distance mask (4 graphs/window)
            for g in range(NW):
                xo_g = xo_sb[:, g, :]
                ptp0 = psum_tp.tile([P, P], dt.float32, tag="ptp")
                nc.tensor.transpose(out=ptp0[:], in_=xo_g[:, 0:P], identity=ident[:])
                xoT0 = s2_pool.tile([P, P], dt.float32, tag="xoT0")
                nc.vector.tensor_copy(xoT0[:], ptp0[:])
                ptp1 = psum_tp.tile([P, P], dt.float32, tag="ptp")
                nc.tensor.transpose(out=ptp1[:], in_=xo_g[:, P:D], identity=ident[:])
                xoT1 = s2_pool.tile([P, P], dt.float32, tag="xoT1")
                nc.vector.tensor_copy(xoT1[:], ptp1[:])

                sq0 = s2_pool.tile([P, P], dt.float32, tag="sq0")
                nc.vector.tensor_tensor(out=sq0[:], in0=xoT0[:], in1=xoT0[:],
                                        op=mybir.AluOpType.mult)
                sq1 = s2_pool.tile([P, P], dt.float32, tag="sq1")
                nc.vector.tensor_tensor(out=sq1[:], in0=xoT1[:], in1=xoT1[:],
                                        op=mybir.AluOpType.mult)
                psq = psum_sq.tile([1, P], dt.float32)
                nc.tensor.matmul(out=psq[:], lhsT=ones_col[:], rhs=sq0[:],
                                 start=True, stop=False)
                nc.tensor.matmul(out=psq[:], lhsT=ones_col[:], rhs=sq1[:],
                                 start=False, stop=True)
                sqrow = s2_pool.tile([1, P], dt.float32, tag="sqrow")
                nc.vector.tensor_copy(sqrow[:], psq[:])

                n2xoT0 = s2_pool.tile([P, P], dt.float32, tag="n2xoT0")
                nc.vector.tensor_scalar(out=n2xoT0[:], in0=xoT0[:], scalar1=-2.0,
                                        scalar2=None, op0=mybir.AluOpType.mult)
                n2xoT1 = s2_pool.tile([P, P], dt.float32, tag="n2xoT1")
                nc.vector.tensor_scalar(out=n2xoT1[:], in0=xoT1[:], scalar1=-2.0,
                                        scalar2=None, op0=mybir.AluOpType.mult)
                ones_row = s2_pool.tile([1, P], dt.float32, tag="ones_row")
                nc.vector.memset(ones_row[:], 1.0)

                adj_g = s2_pool.tile([P, A], dt.uint8, tag="adj")
                for i in range(4):
                    sl = slice(A * i, A * (i + 1))
                    psc = psum_sc.tile([A, A], dt.float32)
                    nc.tensor.matmul(out=psc[:], lhsT=n2xoT0[:, sl], rhs=xoT0[:, sl],
                                     start=True, stop=False)
                    nc.tensor.matmul(out=psc[:], lhsT=n2xoT1[:, sl], rhs=xoT1[:, sl],
                                     start=False, stop=False)
                    nc.tensor.matmul(out=psc[:], lhsT=sqrow[:, sl], rhs=ones_row[:, sl],
                                     start=False, stop=False)
                    nc.tensor.matmul(out=psc[:], lhsT=ones_row[:, sl], rhs=sqrow[:, sl],
                                     start=False, stop=True)
                    smin = s2_pool.tile([A, 1], dt.float32, tag="smin")
                    nc.vector.tensor_reduce(smin[:], psc[:], axis=mybir.AxisListType.X,
                                            op=mybir.AluOpType.min)
                    smax = s2_pool.tile([A, 1], dt.float32, tag="smax")
                    nc.vector.tensor_reduce(smax[:], psc[:], axis=mybir.AxisListType.X,
                                            op=mybir.AluOpType.max)
                    # thr = 0.5*smin + 0.5*smax + 0.5*EPS
                    thr = s2_pool.tile([A, 1], dt.float32, tag="thr")
                    nc.vector.tensor_scalar(out=thr[:], in0=smin[:],
                                            scalar1=0.5, scalar2=0.5 * EPS,
                                            op0=mybir.AluOpType.mult,
                                            op1=mybir.AluOpType.add)
                    nc.vector.scalar_tensor_tensor(out=thr[:], in0=smax[:], scalar=0.5,
                                                   in1=thr[:],
                                                   op0=mybir.AluOpType.mult,
                                                   op1=mybir.AluOpType.add)
                    nc.vector.tensor_scalar(out=adj_g[A * i:A * (i + 1), :],
                                            in0=psc[:], scalar1=thr[:, :1],
                                            scalar2=None,
                                            op0=mybir.AluOpType.is_gt)
                nc.sync.dma_start(adj_out[P * g:P * (g + 1), :], adj_g[:])

    nc.compile()
    return nc


def _prep(X, edge_index, W, b):
    """Host-side sharding/layout prep (index-topology work only)."""
    Xf = np.ascontiguousarray(X, np.float32).reshape(N, D)
    XT = np.ascontiguousarray(Xf.T)                      # [D, N]
    src = edge_index[0].astype(np.int64)
    tgt = edge_index[1].astype(np.int64)
    deg = (np.bincount(tgt, minlength=N) + 1).astype(np.float64)
    dinv = (1.0 / np.sqrt(deg)).astype(np.float32)

    loop = np.arange(N, dtype=np.int64)
    src_all = np.concatenate([src, loop])
    tgt_all = np.concatenate([tgt, loop])

    order = np.argsort(tgt_all, kind="stable")
    s_src = src_all[order]
    s_tgt = tgt_all[order]

    win = s_tgt // WIN                                   # global window id
    counts = np.bincount(win, minlength=NWG)
    CPW = int(np.ceil(counts.max() / 128))
    EPW = CPW * 128

    win_starts = np.cumsum(counts) - counts
    pos = np.arange(len(s_tgt)) - win_starts[win]
    src_pad = np.zeros((NWG, EPW), np.int16)
    rel_pad = np.full((NWG, EPW), -1.0, np.float32)
    src_pad[win, pos] = s_src.astype(np.int16)
    rel_pad[win, pos] = (s_tgt - win * WIN).astype(np.float32)

    b_rep = np.tile(np.asarray(b, np.float32).reshape(1, D), (128, 1))
    dinv_full_t = np.ascontiguousarray(dinv.reshape(N // 128, 128).T)
    Wc = np.ascontiguousarray(W, np.float32)

    in_maps = []
    for c in range(NCORES):
        flat_src = src_pad[c * NW:(c + 1) * NW].reshape(-1)      # [NW*EPW]
        flat_rel = rel_pad[c * NW:(c + 1) * NW].reshape(-1)
        srcw = np.tile(np.ascontiguousarray(flat_src.reshape(-1, 16).T), (8, 1))
        tgtr = np.ascontiguousarray(flat_rel.reshape(NW * CPW, 128).T)
        dinv_t = np.ascontiguousarray(
            dinv[c * TPC:(c + 1) * TPC].reshape(NW, 128).T)
        in_maps.append(dict(XT=XT, Wm=Wc, b_rep=b_rep,
                            dinv_full_t=dinv_full_t, dinv_t=dinv_t,
                            srcw=srcw, tgtr=tgtr))
    return CPW, in_maps


def _run(X, edge_index, W, b):
    from concourse.bass_utils import run_bass_kernel_spmd
    CPW, in_maps = _prep(X, edge_index, W, b)
    if CPW not in _compiled:
        _compiled[CPW] = _build(CPW)
    nc = _compiled[CPW]
    res = run_bass_kernel_spmd(nc, in_maps, core_ids=list(range(NCORES)))
    Xo = np.concatenate([r["Xo_out"] for r in res.results], 0).reshape(B, A, D)
    adj = np.concatenate([r["adj_out"] for r in res.results], 0)
    adj = adj.reshape(B, A, A).astype(bool)
    return Xo, adj


def kernel(X, edge_index, W, b):
    return _run(np.asarray(X), np.asarray(edge_index), np.asarray(W), np.asarray(b))


# revision 6
# speedup vs baseline: 1.1285x; 1.1285x over previous
"""Trainium2 Bass kernel for SingleDGC (GCNConv + per-batch pairwise-distance
adjacency), data-parallel over 8 NeuronCores.

kernel(X, edge_index, W, b) -> (Xo [512,32,256] f32, adj_mask [512,32,32] bool)

Sharding: each core owns a contiguous block of 2048 target nodes (= 64
graphs). Edges are partitioned by target on the host (random edges cross
graph boundaries), sorted by (target-window, source), and padded per
128-target window. On device, each core computes the full XW = X @ W in
fp32 (from a host-transposed XT), scales rows by dinv[src], splits each row
into bf16 hi+lo halves (hi|lo packed per row, lossless to ~2^-17 relative),
and stages Y_hilo to DRAM. Per window, dma_gather pulls the per-edge source
rows (1KB each); a one-hot M matrix built on the vector engine scatters
them into PSUM via a single bf16 [128x128]x[128x512] matmul per chunk
(hi and lo accumulate side by side, summed in the epilogue). Stage 2
computes per-graph pairwise squared distances with an augmented-matmul
trick and thresholds the min-max normalized scores.

Each window's edges are additionally split at source row 8192 so the first
gathers only depend on the first half of Y, overlapping the Q7 descriptor
generation (the critical path: ~8ns/edge) with the XW phase.
"""
import numpy as np

B, A, D = 512, 32, 256
N = B * A                 # 16384 nodes
NCORES = 8
TPC = N // NCORES         # 2048 targets per core
WIN = 128                 # targets per PSUM window
NW = TPC // WIN           # 16 windows per core
NWG = N // WIN            # 128 global windows
EPS = 1e-5
HALF = N // 2             # source split point for XW/gather overlap

_compiled = {}            # (CPA, CPB) -> nc


def _build(CPA, CPB):
    import concourse.bacc as bacc
    import concourse.tile as tile
    from concourse import mybir
    from concourse.masks import make_identity
    import contextlib

    dt = mybir.dt
    P = 128
    CPW = CPA + CPB                   # chunks per window
    NCH = NW * CPW                    # chunks per core
    EPW = CPW * P

    nc = bacc.Bacc("TRN2", target_bir_lowering=False, debug=False,
                   num_devices=NCORES)

    XT = nc.dram_tensor("XT", [D, N], dt.float32, kind="ExternalInput").ap()
    Wm = nc.dram_tensor("Wm", [D, D], dt.float32, kind="ExternalInput").ap()
    b_rep = nc.dram_tensor("b_rep", [P, D], dt.float32, kind="ExternalInput").ap()
    dinv_full_t = nc.dram_tensor("dinv_full_t", [P, N // P], dt.float32,
                                 kind="ExternalInput").ap()
    dinv_t = nc.dram_tensor("dinv_t", [P, NW], dt.float32, kind="ExternalInput").ap()
    srcw = nc.dram_tensor("srcw", [P, NW * EPW // 16], dt.int16,
                          kind="ExternalInput").ap()
    tgtr = nc.dram_tensor("tgtr", [P, NCH], dt.float32, kind="ExternalInput").ap()

    Xo_out = nc.dram_tensor("Xo_out", [TPC, D], dt.float32, kind="ExternalOutput").ap()
    adj_out = nc.dram_tensor("adj_out", [TPC, A], dt.uint8, kind="ExternalOutput").ap()

    with tile.TileContext(nc) as tc:
        with contextlib.ExitStack() as ctx:
            const = ctx.enter_context(tc.tile_pool(name="const", bufs=1))
            xt_pool = ctx.enter_context(tc.tile_pool(name="xt", bufs=3))
            psum_xw = ctx.enter_context(tc.tile_pool(name="pxw", bufs=2, space="PSUM"))
            y_pool = ctx.enter_context(tc.tile_pool(name="y", bufs=3))
            dram = ctx.enter_context(tc.tile_pool(name="dram", bufs=1, space="DRAM"))
            gath_pool = ctx.enter_context(tc.tile_pool(name="gath", bufs=3))
            m_pool = ctx.enter_context(tc.tile_pool(name="m", bufs=4))
            psum_agg = ctx.enter_context(tc.tile_pool(name="pagg", bufs=2, space="PSUM"))
            xo_res = ctx.enter_context(tc.tile_pool(name="xores", bufs=1))
            psum_tp = ctx.enter_context(tc.tile_pool(name="ptp", bufs=2, space="PSUM"))
            psum_sq = ctx.enter_context(tc.tile_pool(name="psq", bufs=1, space="PSUM"))
            psum_sc = ctx.enter_context(tc.tile_pool(name="psc", bufs=1, space="PSUM"))
            s2_pool = ctx.enter_context(tc.tile_pool(name="s2", bufs=2))

            # ---- constants
            w0 = const.tile([P, D], dt.float32)
            nc.sync.dma_start(w0[:], Wm[0:P, :])
            w1 = const.tile([P, D], dt.float32)
            nc.sync.dma_start(w1[:], Wm[P:D, :])
            t_brep = const.tile([P, D], dt.float32)
            nc.sync.dma_start(t_brep[:], b_rep[:])
            t_dft = const.tile([P, N // P], dt.float32)
            nc.sync.dma_start(t_dft[:], dinv_full_t[:])
            t_dt = const.tile([P, NW], dt.float32)
            nc.sync.dma_start(t_dt[:], dinv_t[:])
            t_srcw = const.tile([P, NW * EPW // 16], dt.int16)
            nc.sync.dma_start(t_srcw[:], srcw[:])
            t_tgtr = const.tile([P, NCH], dt.float32)
            nc.sync.dma_start(t_tgtr[:], tgtr[:])
            iota_i = const.tile([P, P], dt.int32)
            nc.gpsimd.iota(iota_i[:], pattern=[[1, P]], base=0, channel_multiplier=0)
            iota_b = const.tile([P, P], dt.bfloat16)
            nc.vector.tensor_copy(iota_b[:], iota_i[:])
            ident = const.tile([P, P], dt.float32)
            make_identity(nc, ident[:])
            ones_col = const.tile([P, 1], dt.float32)
            nc.vector.memset(ones_col[:], 1.0)

            Y_dram = dram.tile([N, 2 * D], dt.bfloat16)   # [hi(256) | lo(256)]

            # ---- phase 1: Y = (X @ W) * dinv[row], split to bf16 hi/lo
            for bt in range(N // 512):
                xt0 = xt_pool.tile([P, 512], dt.float32, tag="xt0")
                nc.sync.dma_start(xt0[:], XT[0:P, 512 * bt:512 * (bt + 1)])
                xt1 = xt_pool.tile([P, 512], dt.float32, tag="xt1")
                nc.sync.dma_start(xt1[:], XT[P:D, 512 * bt:512 * (bt + 1)])
                for j in range(4):
                    t = 4 * bt + j
                    pxw = psum_xw.tile([P, D], dt.float32)
                    nc.tensor.matmul(out=pxw[:], lhsT=xt0[:, P * j:P * (j + 1)],
                                     rhs=w0[:], start=True, stop=False)
                    nc.tensor.matmul(out=pxw[:], lhsT=xt1[:, P * j:P * (j + 1)],
                                     rhs=w1[:], start=False, stop=True)
                    ys = y_pool.tile([P, 2 * D], dt.bfloat16, tag="ys")
                    yhi32 = y_pool.tile([P, D], dt.float32, tag="yhi32")
                    dcol = t_dft[:, t:t + 1]
                    nc.vector.tensor_scalar(out=ys[:, 0:D], in0=pxw[:],
                                            scalar1=dcol, scalar2=None,
                                            op0=mybir.AluOpType.mult)
                    nc.vector.tensor_copy(yhi32[:], ys[:, 0:D])
                    nc.vector.scalar_tensor_tensor(out=ys[:, D:2 * D], in0=pxw[:],
                                                   scalar=dcol, in1=yhi32[:],
                                                   op0=mybir.AluOpType.mult,
                                                   op1=mybir.AluOpType.subtract)
                    nc.sync.dma_start(Y_dram[P * t:P * (t + 1), :], ys[:])

            # ---- phase 2: aggregation per window
            Y_A = Y_dram[0:HALF, :]
            Y_B = Y_dram[HALF:N, :]
            xo_sb = xo_res.tile([P, NW, D], dt.float32)
            for w in range(NW):
                pagg = psum_agg.tile([P, 2 * D], dt.float32)
                for (c0, c1, ysrc) in ((0, CPA, Y_A), (CPA, CPW, Y_B)):
                    ng = c1 - c0
                    gt = gath_pool.tile([P, max(CPA, CPB), 2 * D], dt.bfloat16,
                                        tag="gt")
                    col0 = (w * CPW + c0) * 8
                    nc.gpsimd.dma_gather(gt[:, 0:ng, :], ysrc,
                                         t_srcw[:, col0:col0 + ng * 8],
                                         ng * P, ng * P, 2 * D,
                                         single_packet=False)
                    for c in range(c0, c1):
                        m = m_pool.tile([P, P], dt.bfloat16)
                        ci = w * CPW + c
                        nc.vector.tensor_scalar(out=m[:], in0=iota_b[:],
                                                scalar1=t_tgtr[:, ci:ci + 1],
                                                scalar2=None,
                                                op0=mybir.AluOpType.is_equal)
                        nc.tensor.matmul(out=pagg[:], lhsT=m[:], rhs=gt[:, c - c0, :],
                                         start=(c == 0), stop=(c == CPW - 1))
                # xo = hi*dinv + lo*dinv + b  (one PSUM operand per op)
                hsum = s2_pool.tile([P, D], dt.float32, tag="hsum")
                nc.vector.tensor_scalar(out=hsum[:], in0=pagg[:, 0:D],
                                        scalar1=t_dt[:, w:w + 1], scalar2=None,
                                        op0=mybir.AluOpType.mult)
                hsum2 = s2_pool.tile([P, D], dt.float32, tag="hsum2")
                nc.vector.scalar_tensor_tensor(out=hsum2[:], in0=pagg[:, D:2 * D],
                                               scalar=t_dt[:, w:w + 1],
                                               in1=hsum[:],
                                               op0=mybir.AluOpType.mult,
                                               op1=mybir.AluOpType.add)
                nc.vector.tensor_tensor(out=xo_sb[:, w, :], in0=hsum2[:],
                                        in1=t_brep[:], op=mybir.AluOpType.add)
                nc.sync.dma_start(Xo_out[P * w:P * (w + 1), :], xo_sb[:, w, :])

            # ---- phase 3: per-graph pairwise distance mask (4 graphs/window)
            for g in range(NW):
                xo_g = xo_sb[:, g, :]
                ptp0 = psum_tp.tile([P, P], dt.float32, tag="ptp")
                nc.tensor.transpose(out=ptp0[:], in_=xo_g[:, 0:P], identity=ident[:])
                xoT0 = s2_pool.tile([P, P], dt.float32, tag="xoT0")
                nc.vector.tensor_copy(xoT0[:], ptp0[:])
                ptp1 = psum_tp.tile([P, P], dt.float32, tag="ptp")
                nc.tensor.transpose(out=ptp1[:], in_=xo_g[:, P:D], identity=ident[:])
                xoT1 = s2_pool.tile([P, P], dt.float32, tag="xoT1")
                nc.vector.tensor_copy(xoT1[:], ptp1[:])

                sq0 = s2_pool.tile([P, P], dt.float32, tag="sq0")
                nc.vector.tensor_tensor(out=sq0[:], in0=xoT0[:], in1=xoT0[:],
                                        op=mybir.AluOpType.mult)
                sq1 = s2_pool.tile([P, P], dt.float32, tag="sq1")
                nc.vector.tensor_tensor(out=sq1[:], in0=xoT1[:], in1=xoT1[:],
                                        op=mybir.AluOpType.mult)
                psq = psum_sq.tile([1, P], dt.float32)
                nc.tensor.matmul(out=psq[:], lhsT=ones_col[:], rhs=sq0[:],
                                 start=True, stop=False)
                nc.tensor.matmul(out=psq[:], lhsT=ones_col[:], rhs=sq1[:],
                                 start=False, stop=True)
                sqrow = s2_pool.tile([1, P], dt.float32, tag="sqrow")
                nc.vector.tensor_copy(sqrow[:], psq[:])

                n2xoT0 = s2_pool.tile([P, P], dt.float32, tag="n2xoT0")
                nc.vector.tensor_scalar(out=n2xoT0[:], in0=xoT0[:], scalar1=-2.0,
                                        scalar2=None, op0=mybir.AluOpType.mult)
                n2xoT1 = s2_pool.tile([P, P], dt.float32, tag="n2xoT1")
                nc.vector.tensor_scalar(out=n2xoT1[:], in0=xoT1[:], scalar1=-2.0,
                                        scalar2=None, op0=mybir.AluOpType.mult)
                ones_row = s2_pool.tile([1, P], dt.float32, tag="ones_row")
                nc.vector.memset(ones_row[:], 1.0)

                adj_g = s2_pool.tile([P, A], dt.uint8, tag="adj")
                for i in range(4):
                    sl = slice(A * i, A * (i + 1))
                    psc = psum_sc.tile([A, A], dt.float32)
                    nc.tensor.matmul(out=psc[:], lhsT=n2xoT0[:, sl], rhs=xoT0[:, sl],
                                     start=True, stop=False)
                    nc.tensor.matmul(out=psc[:], lhsT=n2xoT1[:, sl], rhs=xoT1[:, sl],
                                     start=False, stop=False)
                    nc.tensor.matmul(out=psc[:], lhsT=sqrow[:, sl], rhs=ones_row[:, sl],
                                     start=False, stop=False)
                    nc.tensor.matmul(out=psc[:], lhsT=ones_row[:, sl], rhs=sqrow[:, sl],
                                     start=False, stop=True)
                    smin = s2_pool.tile([A, 1], dt.float32, tag="smin")
                    nc.vector.tensor_reduce(smin[:], psc[:], axis=mybir.AxisListType.X,
                                            op=mybir.AluOpType.min)
                    smax = s2_pool.tile([A, 1], dt.float32, tag="smax")
                    nc.vector.tensor_reduce(smax[:], psc[:], axis=mybir.AxisListType.X,
                                            op=mybir.AluOpType.max)
                    # thr = 0.5*smin + 0.5*smax + 0.5*EPS
                    thr = s2_pool.tile([A, 1], dt.float32, tag="thr")
                    nc.vector.tensor_scalar(out=thr[:], in0=smin[:],
                                            scalar1=0.5, scalar2=0.5 * EPS,
                                            op0=mybir.AluOpType.mult,
                                            op1=mybir.AluOpType.add)
                    nc.vector.scalar_tensor_tensor(out=thr[:], in0=smax[:], scalar=0.5,
                                                   in1=thr[:],
                                                   op0=mybir.AluOpType.mult,
                                                   op1=mybir.AluOpType.add)
                    nc.vector.tensor_scalar(out=adj_g[A * i:A * (i + 1), :],
                                            in0=psc[:], scalar1=thr[:, :1],
                                            scalar2=None,
                                            op0=mybir.AluOpType.is_gt)
                nc.sync.dma_start(adj_out[P * g:P * (g + 1), :], adj_g[:])

    nc.compile()
    return nc


def _prep(X, edge_index, W, b):
    """Host-side sharding/layout prep (index-topology work only)."""
    import ml_dtypes
    Xf = np.ascontiguousarray(X, np.float32).reshape(N, D)
    XT = np.ascontiguousarray(Xf.T)                      # [D, N]
    src = edge_index[0].astype(np.int64)
    tgt = edge_index[1].astype(np.int64)
    deg = (np.bincount(tgt, minlength=N) + 1).astype(np.float64)
    dinv = (1.0 / np.sqrt(deg)).astype(np.float32)

    loop = np.arange(N, dtype=np.int64)
    src_all = np.concatenate([src, loop])
    tgt_all = np.concatenate([tgt, loop])

    # sort by (window, src-half) so each window's chunks split cleanly at HALF
    win = tgt_all // WIN
    half = (src_all >= HALF).astype(np.int64)
    order = np.lexsort((src_all, half, win))
    s_src = src_all[order]
    s_tgt = tgt_all[order]
    s_win = win[order]
    s_half = half[order]

    # per (window, half) counts -> global chunk capacities CPA / CPB
    wh = s_win * 2 + s_half
    counts = np.bincount(wh, minlength=2 * NWG)
    CPA = int(np.ceil(counts[0::2].max() / 128))
    CPB = int(np.ceil(counts[1::2].max() / 128))
    CPW = CPA + CPB
    EPW = CPW * 128

    # slot position within padded [window][A-part | B-part] layout
    wh_starts = np.cumsum(counts) - counts
    pos_in_grp = np.arange(len(s_tgt)) - wh_starts[wh]
    slot = s_win * EPW + s_half * (CPA * 128) + pos_in_grp

    src_pad = np.zeros(NWG * EPW, np.int16)
    rel_pad = np.full(NWG * EPW, -1.0, np.float32)
    # B-group indices are relative to Y[HALF:]
    src_pad[slot] = (s_src - s_half * HALF).astype(np.int16)
    rel_pad[slot] = (s_tgt - s_win * WIN).astype(np.float32)
    src_pad = src_pad.reshape(NWG, EPW)
    rel_pad = rel_pad.reshape(NWG, EPW)

    b_rep = np.tile(np.asarray(b, np.float32).reshape(1, D), (128, 1))
    dinv_full_t = np.ascontiguousarray(dinv.reshape(N // 128, 128).T)
    Wc = np.ascontiguousarray(W, np.float32)

    in_maps = []
    for c in range(NCORES):
        flat_src = src_pad[c * NW:(c + 1) * NW].reshape(-1)      # [NW*EPW]
        flat_rel = rel_pad[c * NW:(c + 1) * NW].reshape(-1)
        srcw = np.tile(np.ascontiguousarray(flat_src.reshape(-1, 16).T), (8, 1))
        tgtr = np.ascontiguousarray(flat_rel.reshape(NW * CPW, 128).T)
        dinv_t = np.ascontiguousarray(
            dinv[c * TPC:(c + 1) * TPC].reshape(NW, 128).T)
        in_maps.append(dict(XT=XT, Wm=Wc, b_rep=b_rep,
                            dinv_full_t=dinv_full_t, dinv_t=dinv_t,
                            srcw=srcw, tgtr=tgtr))
    return (CPA, CPB), in_maps


def _run(X, edge_index, W, b):
    from concourse.bass_utils import run_bass_kernel_spmd
    key, in_maps = _prep(X, edge_index, W, b)
    if key not in _compiled:
        _compiled[key] = _build(*key)
    nc = _compiled[key]
    res = run_bass_kernel_spmd(nc, in_maps, core_ids=list(range(NCORES)))
    Xo = np.concatenate([r["Xo_out"] for r in res.results], 0).reshape(B, A, D)
    adj = np.concatenate([r["adj_out"] for r in res.results], 0)
    adj = adj.reshape(B, A, A).astype(bool)
    return Xo, adj


def kernel(X, edge_index, W, b):
    return _run(np.asarray(X), np.asarray(edge_index), np.asarray(W), np.asarray(b))


# revision 7
# speedup vs baseline: 1.3563x; 1.2018x over previous
"""Trainium2 Bass kernel for SingleDGC (GCNConv + per-batch pairwise-distance
adjacency), data-parallel over 8 NeuronCores.

kernel(X, edge_index, W, b) -> (Xo [512,32,256] f32, adj_mask [512,32,32] bool)

Design notes:
- Each core owns 2048 contiguous target nodes (64 graphs). Edges are
  partitioned by target window on the host, sorted by (window, source-half),
  padded per 128-target window, and gathered per-edge with dma_gather.
- Critical path is the Q7 SWDGE descriptor generation (~8ns/edge). All DVE
  work is shaped to avoid the DVE<->GpSimd shared SBUF port (PSUM-sourced
  in0, broadcast SBUF in1 on the dedicated read port, no AP-scalar reads in
  hot ops) so it runs concurrently with descriptor generation.
- XW = X @ W runs in fp32; Y rows are scaled by dinv[src] and split into
  bf16 hi+lo (relative error ~2^-17), so each 128-edge chunk scatters with a
  single bf16 [128x128] x [128x512] matmul (hi and lo side by side in PSUM).
- The bias is folded into the accumulation as a K=1 matmul of
  sqrt(deg)[tgt] x (b_hi|b_lo), so the dinv post-scale reproduces +b.
- Stage 2 squared distances use an augmented matmul: -2*x_i.x_j via two
  K=128 bf-free fp32 matmuls, +sq_i and +sq_j via K=1 rank-1 matmuls.
"""
import numpy as np

B, A, D = 512, 32, 256
N = B * A                 # 16384 nodes
NCORES = 8
TPC = N // NCORES         # 2048 targets per core
WIN = 128                 # targets per PSUM window
NW = TPC // WIN           # 16 windows per core
NWG = N // WIN            # 128 global windows
EPS = 1e-5
HALF = N // 2             # source split point for XW/gather overlap

_compiled = {}            # (CPA, CPB) -> nc


def _build(CPA, CPB):
    import concourse.bacc as bacc
    import concourse.tile as tile
    from concourse import mybir
    from concourse.masks import make_identity
    import contextlib

    dt = mybir.dt
    P = 128
    CPW = CPA + CPB                   # chunks per window
    NCH = NW * CPW                    # chunks per core
    EPW = CPW * P

    nc = bacc.Bacc("TRN2", target_bir_lowering=False, debug=False,
                   num_devices=NCORES)

    XT = nc.dram_tensor("XT", [D, N], dt.float32, kind="ExternalInput").ap()
    Wm = nc.dram_tensor("Wm", [D, D], dt.float32, kind="ExternalInput").ap()
    dinv_full_t = nc.dram_tensor("dinv_full_t", [P, N // P], dt.float32,
                                 kind="ExternalInput").ap()
    dinv_t = nc.dram_tensor("dinv_t", [P, NW], dt.float32, kind="ExternalInput").ap()
    srcw = nc.dram_tensor("srcw", [P, NW * EPW // 16], dt.int16,
                          kind="ExternalInput").ap()
    tgtr = nc.dram_tensor("tgtr", [P, NCH], dt.float32, kind="ExternalInput").ap()
    sqdeg = nc.dram_tensor("sqdeg", [1, TPC], dt.bfloat16, kind="ExternalInput").ap()
    b_hilo = nc.dram_tensor("b_hilo", [1, 2 * D], dt.bfloat16,
                            kind="ExternalInput").ap()

    Xo_out = nc.dram_tensor("Xo_out", [TPC, D], dt.float32, kind="ExternalOutput").ap()
    adj_out = nc.dram_tensor("adj_out", [TPC, A], dt.uint8, kind="ExternalOutput").ap()

    with tile.TileContext(nc) as tc:
        with contextlib.ExitStack() as ctx:
            const = ctx.enter_context(tc.tile_pool(name="const", bufs=1))
            xt_pool = ctx.enter_context(tc.tile_pool(name="xt", bufs=3))
            y_pool = ctx.enter_context(tc.tile_pool(name="y", bufs=3))
            dram = ctx.enter_context(tc.tile_pool(name="dram", bufs=1, space="DRAM"))
            gath_pool = ctx.enter_context(tc.tile_pool(name="gath", bufs=3))
            m_pool = ctx.enter_context(tc.tile_pool(name="m", bufs=4))
            s2_pool = ctx.enter_context(tc.tile_pool(name="s2", bufs=2))
            xo_res = ctx.enter_context(tc.tile_pool(name="xores", bufs=1))
            # PSUM: mix(2) + pagg(2) + piota(1) + ptp(2) + psq(1) = 8 banks
            mix = ctx.enter_context(tc.tile_pool(name="mix", bufs=2, space="PSUM"))
            psum_agg = ctx.enter_context(tc.tile_pool(name="pagg", bufs=2, space="PSUM"))
            piota = ctx.enter_context(tc.tile_pool(name="piota", bufs=1, space="PSUM"))
            psum_tp = ctx.enter_context(tc.tile_pool(name="ptp", bufs=2, space="PSUM"))
            psum_sq = ctx.enter_context(tc.tile_pool(name="psq", bufs=1, space="PSUM"))

            # ---- constants
            w0 = const.tile([P, D], dt.float32)
            nc.sync.dma_start(w0[:], Wm[0:P, :])
            w1 = const.tile([P, D], dt.float32)
            nc.sync.dma_start(w1[:], Wm[P:D, :])
            t_dft = const.tile([P, N // P], dt.float32)
            nc.sync.dma_start(t_dft[:], dinv_full_t[:])
            t_dt = const.tile([P, NW], dt.float32)
            nc.sync.dma_start(t_dt[:], dinv_t[:])
            t_srcw = const.tile([P, NW * EPW // 16], dt.int16)
            nc.sync.dma_start(t_srcw[:], srcw[:])
            t_tgtr = const.tile([P, NCH], dt.float32)
            nc.sync.dma_start(t_tgtr[:], tgtr[:])
            t_sqdeg = const.tile([1, TPC], dt.bfloat16)
            nc.sync.dma_start(t_sqdeg[:], sqdeg[:])
            t_bhl = const.tile([1, 2 * D], dt.bfloat16)
            nc.sync.dma_start(t_bhl[:], b_hilo[:])
            iota_i = const.tile([P, P], dt.int32)
            nc.gpsimd.iota(iota_i[:], pattern=[[1, P]], base=0, channel_multiplier=0)
            iota_ps = piota.tile([P, P], dt.float32)
            nc.vector.tensor_copy(iota_ps[:], iota_i[:])
            ident = const.tile([P, P], dt.float32)
            make_identity(nc, ident[:])
            ones_col = const.tile([P, 1], dt.float32)
            nc.vector.memset(ones_col[:], 1.0)

            Y_dram = dram.tile([N, 2 * D], dt.bfloat16)   # [hi(256) | lo(256)]

            # ---- phase 1: Y = (X @ W) * dinv[row], split to bf16 hi/lo
            for bt in range(N // 512):
                xt0 = xt_pool.tile([P, 512], dt.float32, tag="xt0")
                nc.sync.dma_start(xt0[:], XT[0:P, 512 * bt:512 * (bt + 1)])
                xt1 = xt_pool.tile([P, 512], dt.float32, tag="xt1")
                nc.sync.dma_start(xt1[:], XT[P:D, 512 * bt:512 * (bt + 1)])
                for j in range(4):
                    t = 4 * bt + j
                    pxw = mix.tile([P, 2 * D], dt.float32, tag="mix")
                    nc.tensor.matmul(out=pxw[:, 0:D], lhsT=xt0[:, P * j:P * (j + 1)],
                                     rhs=w0[:], start=True, stop=False)
                    nc.tensor.matmul(out=pxw[:, 0:D], lhsT=xt1[:, P * j:P * (j + 1)],
                                     rhs=w1[:], start=False, stop=True)
                    dbc = t_dft[:, t:t + 1].to_broadcast([P, D])
                    # y (f32, exact) staged in the same PSUM bank
                    nc.vector.tensor_tensor(out=pxw[:, D:2 * D], in0=pxw[:, 0:D],
                                            in1=dbc, op=mybir.AluOpType.mult)
                    ys = y_pool.tile([P, 2 * D], dt.bfloat16, tag="ys")
                    nc.vector.tensor_copy(ys[:, 0:D], pxw[:, D:2 * D])
                    nc.vector.tensor_tensor(out=ys[:, D:2 * D], in0=pxw[:, D:2 * D],
                                            in1=ys[:, 0:D],
                                            op=mybir.AluOpType.subtract)
                    nc.sync.dma_start(Y_dram[P * t:P * (t + 1), :], ys[:])

            # ---- phase 2: aggregation per window
            Y_A = Y_dram[0:HALF, :]
            Y_B = Y_dram[HALF:N, :]
            xo_sb = xo_res.tile([P, NW, D], dt.float32)
            for w in range(NW):
                pagg = psum_agg.tile([P, 2 * D], dt.float32)
                for (c0, c1, ysrc) in ((0, CPA, Y_A), (CPA, CPW, Y_B)):
                    ng = c1 - c0
                    gt = gath_pool.tile([P, max(CPA, CPB), 2 * D], dt.bfloat16,
                                        tag="gt")
                    col0 = (w * CPW + c0) * 8
                    nc.gpsimd.dma_gather(gt[:, 0:ng, :], ysrc,
                                         t_srcw[:, col0:col0 + ng * 8],
                                         ng * P, ng * P, 2 * D,
                                         single_packet=False)
                    for c in range(c0, c1):
                        m = m_pool.tile([P, P], dt.bfloat16)
                        ci = w * CPW + c
                        nc.vector.tensor_tensor(
                            out=m[:], in0=iota_ps[:],
                            in1=t_tgtr[:, ci:ci + 1].to_broadcast([P, P]),
                            op=mybir.AluOpType.is_equal)
                        nc.tensor.matmul(out=pagg[:], lhsT=m[:], rhs=gt[:, c - c0, :],
                                         start=(c == 0), stop=False)
                # bias term: += sqdeg[tgt] (x) (b_hi | b_lo), K=1
                nc.tensor.matmul(out=pagg[:],
                                 lhsT=t_sqdeg[:, P * w:P * (w + 1)],
                                 rhs=t_bhl[:], start=False, stop=True)
                # xo = (hi + lo) * dinv[tgt]
                s1 = s2_pool.tile([P, D], dt.float32, tag="s1")
                nc.vector.tensor_copy(s1[:], pagg[:, 0:D])
                nc.vector.tensor_tensor(out=pagg[:, 0:D], in0=pagg[:, D:2 * D],
                                        in1=s1[:], op=mybir.AluOpType.add)
                nc.vector.tensor_tensor(
                    out=xo_sb[:, w, :], in0=pagg[:, 0:D],
                    in1=t_dt[:, w:w + 1].to_broadcast([P, D]),
                    op=mybir.AluOpType.mult)
                nc.sync.dma_start(Xo_out[P * w:P * (w + 1), :], xo_sb[:, w, :])

            # ---- phase 3: per-graph pairwise distance mask (4 graphs/window)
            for g in range(NW):
                xo_g = xo_sb[:, g, :]
                ptp0 = psum_tp.tile([P, P], dt.float32, tag="ptp")
                nc.tensor.transpose(out=ptp0[:], in_=xo_g[:, 0:P], identity=ident[:])
                xoT0 = s2_pool.tile([P, P], dt.float32, tag="xoT0")
                nc.vector.tensor_copy(xoT0[:], ptp0[:])
                ptp1 = psum_tp.tile([P, P], dt.float32, tag="ptp")
                nc.tensor.transpose(out=ptp1[:], in_=xo_g[:, P:D], identity=ident[:])
                xoT1 = s2_pool.tile([P, P], dt.float32, tag="xoT1")
                nc.vector.tensor_copy(xoT1[:], ptp1[:])

                # squares read the PSUM copy (keeps DVE off the shared port)
                sq0 = s2_pool.tile([P, P], dt.float32, tag="sq0")
                nc.vector.tensor_tensor(out=sq0[:], in0=ptp0[:], in1=xoT0[:],
                                        op=mybir.AluOpType.mult)
                n2xoT0 = s2_pool.tile([P, P], dt.float32, tag="n2xoT0")
                nc.vector.tensor_scalar(out=n2xoT0[:], in0=ptp0[:], scalar1=-2.0,
                                        scalar2=None, op0=mybir.AluOpType.mult)
                sq1 = s2_pool.tile([P, P], dt.float32, tag="sq1")
                nc.vector.tensor_tensor(out=sq1[:], in0=ptp1[:], in1=xoT1[:],
                                        op=mybir.AluOpType.mult)
                n2xoT1 = s2_pool.tile([P, P], dt.float32, tag="n2xoT1")
                nc.vector.tensor_scalar(out=n2xoT1[:], in0=ptp1[:], scalar1=-2.0,
                                        scalar2=None, op0=mybir.AluOpType.mult)

                psq = psum_sq.tile([1, P], dt.float32)
                nc.tensor.matmul(out=psq[:], lhsT=ones_col[:], rhs=sq0[:],
                                 start=True, stop=False)
                nc.tensor.matmul(out=psq[:], lhsT=ones_col[:], rhs=sq1[:],
                                 start=False, stop=True)
                sqrow = s2_pool.tile([1, P], dt.float32, tag="sqrow")
                nc.vector.tensor_copy(sqrow[:], psq[:])
                ones_row = s2_pool.tile([1, P], dt.float32, tag="ones_row")
                nc.vector.memset(ones_row[:], 1.0)

                adj_g = s2_pool.tile([P, A], dt.uint8, tag="adj")
                for i in range(4):
                    sl = slice(A * i, A * (i + 1))
                    psc = mix.tile([A, A], dt.float32, tag="mix")
                    nc.tensor.matmul(out=psc[:], lhsT=n2xoT0[:, sl], rhs=xoT0[:, sl],
                                     start=True, stop=False)
                    nc.tensor.matmul(out=psc[:], lhsT=n2xoT1[:, sl], rhs=xoT1[:, sl],
                                     start=False, stop=False)
                    nc.tensor.matmul(out=psc[:], lhsT=sqrow[:, sl], rhs=ones_row[:, sl],
                                     start=False, stop=False)
                    nc.tensor.matmul(out=psc[:], lhsT=ones_row[:, sl], rhs=sqrow[:, sl],
                                     start=False, stop=True)
                    smin = s2_pool.tile([A, 1], dt.float32, tag="smin")
                    nc.vector.tensor_reduce(smin[:], psc[:], axis=mybir.AxisListType.X,
                                            op=mybir.AluOpType.min)
                    smax_ps = mix.tile([A, 1], dt.float32, tag="mix")
                    nc.vector.tensor_reduce(smax_ps[:], psc[:],
                                            axis=mybir.AxisListType.X,
                                            op=mybir.AluOpType.max)
                    # thr = 0.5*smin + 0.5*EPS + 0.5*smax
                    thr = s2_pool.tile([A, 1], dt.float32, tag="thr")
                    nc.vector.tensor_scalar(out=thr[:], in0=smin[:],
                                            scalar1=0.5, scalar2=0.5 * EPS,
                                            op0=mybir.AluOpType.mult,
                                            op1=mybir.AluOpType.add)
                    nc.vector.scalar_tensor_tensor(out=thr[:], in0=smax_ps[:],
                                                   scalar=0.5, in1=thr[:],
                                                   op0=mybir.AluOpType.mult,
                                                   op1=mybir.AluOpType.add)
                    nc.vector.tensor_tensor(out=adj_g[A * i:A * (i + 1), :],
                                            in0=psc[:],
                                            in1=thr[:, :1].to_broadcast([A, A]),
                                            op=mybir.AluOpType.is_gt)
                nc.sync.dma_start(adj_out[P * g:P * (g + 1), :], adj_g[:])

    nc.compile()
    return nc


def _prep(X, edge_index, W, b):
    """Host-side sharding/layout prep (index-topology work only)."""
    import ml_dtypes
    Xf = np.ascontiguousarray(X, np.float32).reshape(N, D)
    XT = np.ascontiguousarray(Xf.T)                      # [D, N]
    src = edge_index[0].astype(np.int64)
    tgt = edge_index[1].astype(np.int64)
    deg = (np.bincount(tgt, minlength=N) + 1).astype(np.float64)
    dinv = (1.0 / np.sqrt(deg)).astype(np.float32)
    sqdeg_full = np.sqrt(deg).astype(np.float32)

    loop = np.arange(N, dtype=np.int64)
    src_all = np.concatenate([src, loop])
    tgt_all = np.concatenate([tgt, loop])

    # sort by (window, src-half) so each window's chunks split cleanly at HALF
    win = tgt_all // WIN
    half = (src_all >= HALF).astype(np.int64)
    order = np.lexsort((src_all, half, win))
    s_src = src_all[order]
    s_tgt = tgt_all[order]
    s_win = win[order]
    s_half = half[order]

    wh = s_win * 2 + s_half
    counts = np.bincount(wh, minlength=2 * NWG)
    CPA = int(np.ceil(counts[0::2].max() / 128))
    CPB = int(np.ceil(counts[1::2].max() / 128))
    CPW = CPA + CPB
    EPW = CPW * 128

    wh_starts = np.cumsum(counts) - counts
    pos_in_grp = np.arange(len(s_tgt)) - wh_starts[wh]
    slot = s_win * EPW + s_half * (CPA * 128) + pos_in_grp

    src_pad = np.zeros(NWG * EPW, np.int16)
    rel_pad = np.full(NWG * EPW, -1.0, np.float32)
    src_pad[slot] = (s_src - s_half * HALF).astype(np.int16)
    rel_pad[slot] = (s_tgt - s_win * WIN).astype(np.float32)
    src_pad = src_pad.reshape(NWG, EPW)
    rel_pad = rel_pad.reshape(NWG, EPW)

    dinv_full_t = np.ascontiguousarray(dinv.reshape(N // 128, 128).T)
    Wc = np.ascontiguousarray(W, np.float32)
    bf = np.asarray(b, np.float32).reshape(1, D)
    b_hi = bf.astype(ml_dtypes.bfloat16)
    b_lo = (bf - b_hi.astype(np.float32)).astype(ml_dtypes.bfloat16)
    b_hilo = np.concatenate([b_hi, b_lo], 1)             # [1, 512]

    in_maps = []
    for c in range(NCORES):
        flat_src = src_pad[c * NW:(c + 1) * NW].reshape(-1)      # [NW*EPW]
        flat_rel = rel_pad[c * NW:(c + 1) * NW].reshape(-1)
        srcw = np.tile(np.ascontiguousarray(flat_src.reshape(-1, 16).T), (8, 1))
        tgtr = np.ascontiguousarray(flat_rel.reshape(NW * CPW, 128).T)
        dinv_t = np.ascontiguousarray(
            dinv[c * TPC:(c + 1) * TPC].reshape(NW, 128).T)
        sqdeg = sqdeg_full[c * TPC:(c + 1) * TPC].reshape(1, TPC) \
            .astype(ml_dtypes.bfloat16)
        in_maps.append(dict(XT=XT, Wm=Wc, b_hilo=b_hilo,
                            dinv_full_t=dinv_full_t, dinv_t=dinv_t,
                            srcw=srcw, tgtr=tgtr, sqdeg=sqdeg))
    return (CPA, CPB), in_maps


def _run(X, edge_index, W, b):
    from concourse.bass_utils import run_bass_kernel_spmd
    key, in_maps = _prep(X, edge_index, W, b)
    if key not in _compiled:
        _compiled[key] = _build(*key)
    nc = _compiled[key]
    res = run_bass_kernel_spmd(nc, in_maps, core_ids=list(range(NCORES)))
    Xo = np.concatenate([r["Xo_out"] for r in res.results], 0).reshape(B, A, D)
    adj = np.concatenate([r["adj_out"] for r in res.results], 0)
    adj = adj.reshape(B, A, A).astype(bool)
    return Xo, adj


def kernel(X, edge_index, W, b):
    return _run(np.asarray(X), np.asarray(edge_index), np.asarray(W), np.asarray(b))
